# revision 28
# baseline (speedup 1.0000x reference)
"""MultiLinearUpsampling Trainium2 kernel.

Problem: out[b, t, :] = W[lidx[t]] @ pooled[b, segc[t], :]  (zero for invalid t)
where segc/lidx derive from sorted pooling_indices (ragged segments).

Strategy
--------
Host computes the segment structure.  Only sum_l N_l matvecs are unique
per batch (N_l = #segments with len > l; positions past offset L-1 in a
segment reuse the l=L-1 result).  The device runs one SPMD program on 8
cores: P phases, phase p = one stationary weight plane (per-core data)
applied to C_p activation columns (per-core data, host-gathered).  A
small packing optimizer cuts the 16 linears' column sets into <=8
pieces per phase to minimize sum(C_p) (the per-core compute).  Matmuls
run in fp16 (full PE rate, ~3e-4 rel err for this data, half the DMA of
fp32); accumulation is fp32 in PSUM and outputs are fp32.  The host
scatters computed vectors to their t positions (including the l=15 tail
replication) and zero-fills invalid t.
"""

from contextlib import ExitStack

import numpy as np

import concourse.bass as bass  # noqa: F401  (bass types via bacc)
import concourse.mybir as mybir
import concourse.tile as tile
from concourse import bacc
from concourse.bass_utils import run_bass_kernel_spmd

F32 = mybir.dt.float32
F16 = mybir.dt.float16

B = 8          # batch (each core sees all batches)
N = 512        # segments
D = 1024       # D_in == D_out
L = 16         # linears
NCORES = 8
KC = 8         # contraction chunks of 128
MC = 8         # output-dim chunks of 128


# ---------------------------------------------------------------------------
# packing: choose phase sizes + piece assignment
# ---------------------------------------------------------------------------

def _combo_dp(sizes, Cs):
    """Assign each item (size) a piece-count vector over phase capacities Cs
    (max 8 pieces per phase) minimizing nothing fancy -- returns None if
    infeasible, else list of per-item count tuples."""
    P = len(Cs)
    items = list(sizes)
    combos_per_item = []
    for sz in items:
        combos = []
        max_counts = [min(8, -(-sz // c) if c else 0) for c in Cs]
        # enumerate small count vectors (total pieces <= 4)
        def rec(i, vec, cap):
            if sum(vec) > 4:
                return
            if i == P:
                if cap >= sz and sum(vec) > 0:
                    combos.append(tuple(vec))
                return
            for n in range(0, min(max_counts[i], 4) + 1):
                rec(i + 1, vec + [n], cap + n * Cs[i])

        rec(0, [], 0)
        if sz > 0 and not combos:
            return None
        combos_per_item.append(combos if sz > 0 else [tuple([0] * P)])

    # DP over cumulative piece counts in phases 0..P-2, minimize last phase
    from functools import lru_cache

    states = {tuple([0] * (P - 1)): 0}
    choice = []
    for combos in combos_per_item:
        nstates = {}
        back = {}
        for st, lastc in states.items():
            for cb in combos:
                nst = tuple(st[i] + cb[i] for i in range(P - 1))
                if any(v > 8 for v in nst):
                    continue
                nl = lastc + cb[P - 1]
                if nl > 8:
                    continue
                if nst not in nstates or nl < nstates[nst]:
                    nstates[nst] = nl
                    back[nst] = (st, cb)
        if not nstates:
            return None
        choice.append(back)
        states = nstates

    st = min(states, key=lambda s: states[s])
    picks = [None] * len(items)
    for i in range(len(items) - 1, -1, -1):
        st_prev, cb = choice[i][st]
        picks[i] = cb
        st = st_prev
    return picks


def _plan(N_l):
    """Return (Cs, slot_map): phase sizes and slot_map[c][p] =
    (l, col_start, cnt) or None."""
    order_l = np.argsort(-np.asarray(N_l), kind="stable")
    Ns = [int(N_l[i]) for i in order_l]
    total = sum(Ns)
    if total == 0:
        return [2], [[None] for _ in range(NCORES)]

    best = None  # (sumC, Cs, picks)

    def consider(Cs):
        nonlocal best
        Cs = [int(c) for c in Cs if c > 0]
        if not Cs:
            return
        if best is not None and sum(Cs) >= best[0]:
            return
        picks = _combo_dp(Ns, Cs)
        if picks is not None:
            best = (sum(Cs), Cs, picks)

    # baseline: heads unsplit at N(0), tails at N(8)
    c2 = Ns[8] if len(Ns) > 8 else 0
    consider([Ns[0], c2] if c2 else [Ns[0]])

    # precomputed optima for the benchmark's N_l (verified for the actual
    # data by the feasibility DP; harmless no-ops when infeasible)
    consider([214, 170, 110])
    consider([158, 132, 94, 80])

    if c2:
        head = Ns[:8]
        # family: tail phase at N(8); two head phases (X, Y) searched
        hi = head[0]
        for X in range(max(hi // 2, 64), hi + 1, 4):
            # minimal Y so every head item fits in <=3 pieces approx
            for Y in range(16, X + 1, 4):
                if 8 * (X + Y) < sum(head):
                    continue
                if best is not None and X + Y + c2 >= best[0]:
                    continue
                picks = _combo_dp(head, [X, Y])
                if picks is None:
                    continue
                # combine with tail phase
                consider([X, Y, c2])
                break  # smallest feasible Y for this X

    Cs = best[1]
    picks = best[2]
    # build slot map: phase -> list of pieces
    P = len(Cs)
    phase_pieces = [[] for _ in range(P)]
    for idx, l in enumerate(order_l):
        sz = Ns[idx]
        if sz == 0:
            continue
        pos = 0
        cb = picks[idx] if len(picks[idx]) == P else tuple(
            list(picks[idx]) + [0] * (P - len(picks[idx]))
        )
        for p in range(P):
            for _ in range(cb[p]):
                cnt = min(Cs[p], sz - pos)
                if cnt <= 0:
                    continue
                phase_pieces[p].append((int(l), pos, cnt))
                pos += cnt
        assert pos >= sz, f"l={l} not covered: {pos}/{sz}"

    slot_map = [[None] * P for _ in range(NCORES)]
    for p in range(P):
        assert len(phase_pieces[p]) <= NCORES, (p, phase_pieces[p])
        for c, piece in enumerate(phase_pieces[p]):
            slot_map[c][p] = piece
    return Cs, slot_map


# ---------------------------------------------------------------------------
# device program
# ---------------------------------------------------------------------------

def _build_program(Cs):
    """Inputs: xt (D, B, CT) f16, wt (P, D, D) f16 (wt[p] = W-plane.T).
    Outputs: y{p} (D, B, C_p) f32."""
    nc = bacc.Bacc("TRN2", target_bir_lowering=False, debug=False)
    P = len(Cs)
    CT = sum(Cs)

    xs = [
        nc.dram_tensor(f"x{p}", (D, B, C), F16, kind="ExternalInput")
        for p, C in enumerate(Cs)
    ]
    wt = nc.dram_tensor("wt", (P, D, D), F16, kind="ExternalInput")
    ys = [
        nc.dram_tensor(f"y{p}", (B, D, C), F32, kind="ExternalOutput")
        for p, C in enumerate(Cs)
    ]

    # (kp, kc, b*c) views: per-k rows are contiguous B*C_p runs -> 2D DMAs
    xs_r = [
        x.ap().rearrange("(kc kp) b n -> kp kc (b n)", kp=128) for x in xs
    ]
    wt_r = wt.ap().rearrange("p (kc kp) m -> kp p kc m", kp=128)

    # process phases smallest-first: the first phase's inputs arrive
    # quickly, later phases' inputs stream in behind its compute
    order_p = sorted(range(P), key=lambda p: Cs[p])

    with tile.TileContext(nc) as tc, ExitStack() as ctx:
        wpool = ctx.enter_context(tc.tile_pool(name="w", bufs=1))
        xpool = ctx.enter_context(tc.tile_pool(name="x", bufs=1))
        opool = ctx.enter_context(tc.tile_pool(name="o", bufs=3))
        ppool = ctx.enter_context(tc.tile_pool(name="ps", bufs=6, space="PSUM"))

        # resident inputs, emitted in consumption order (few large DMAs:
        # each dma_start costs ~600ns of sequencer issue time regardless
        # of size, so instruction count matters more than granularity)
        wtiles = {}
        xtiles = {}
        for p in order_p:
            C = Cs[p]
            for k in range(KC):
                wtiles[p, k] = wpool.tile(
                    [128, D], F16, tag=f"w{p}_{k}", name=f"w{p}_{k}"
                )
                nc.sync.dma_start(wtiles[p, k][:], wt_r[:, p, k])
                xtiles[p, k] = xpool.tile(
                    [128, B, C], F16, tag=f"x{p}_{k}", name=f"x{p}_{k}"
                )
                nc.sync.dma_start(
                    xtiles[p, k][:].rearrange("kp b n -> kp (b n)"), xs_r[p][:, k]
                )

        for p in order_p:
            C = Cs[p]
            g = max(1, min(B, 512 // C))
            for b0 in range(0, B, g):
                gg = min(g, B - b0)
                ot = opool.tile([128, MC, gg, C], F32, tag="o")
                for m in range(MC):
                    ps = ppool.tile([128, gg, C], F32, tag="ps")
                    for k in range(KC):
                        nc.tensor.matmul(
                            ps[:],
                            wtiles[p, k][:, m * 128 : (m + 1) * 128],
                            xtiles[p, k][:, b0 : b0 + gg, :],
                            start=(k == 0),
                            stop=(k == KC - 1),
                        )
                    nc.vector.tensor_copy(ot[:, m], ps[:])
                # one output DMA per batch row (GpSimd queue: keeps the
                # Sync sequencer free for input issue)
                for bi in range(gg):
                    nc.gpsimd.dma_start(
                        ys[p][b0 + bi].rearrange("(m kp) c -> kp m c", kp=128),
                        ot[:, :, bi, :],
                    )

    nc.compile()
    return nc


# ---------------------------------------------------------------------------
# host wrapper
# ---------------------------------------------------------------------------

def _segment_structure(idx, T):
    t = np.arange(T)
    seg = np.searchsorted(idx, t, side="left")
    valid = seg < N
    segc = np.clip(seg, 0, N - 1)
    start = np.where(segc > 0, idx[np.maximum(segc - 1, 0)] + 1, 0)
    lidx = np.minimum(t - start, L - 1).astype(np.int64)
    lens = np.bincount(segc[valid], minlength=N)
    return t, seg, valid, segc, lidx, lens


def _install_ntff_hook():
    """Profiling-only: register the axon NTFF profile hook (dev use)."""
    import sys
    import types

    try:
        import antenv

        if "antenv.axon_hooks" not in sys.modules:
            mod = types.ModuleType("antenv.axon_hooks")
            holder = [None]
            mod.set_axon_ntff_profile_hook = lambda h: holder.__setitem__(0, h)
            mod.get_axon_ntff_profile_hook = lambda: holder[0]
            sys.modules["antenv.axon_hooks"] = mod
            antenv.axon_hooks = mod
            from trn_agent_boot.trn_boot import _ntff_profile_via_ctypes

            mod.set_axon_ntff_profile_hook(
                _ntff_profile_via_ctypes("/opt/axon/libaxon_pjrt.so")
            )
    except Exception as e:
        print(f"NTFF hook install failed: {e}")


def kernel(pooled_vectors, W, pooling_indices, target_length, _trace=False):
    pooled = np.asarray(pooled_vectors, dtype=np.float32)
    Wf = np.asarray(W, dtype=np.float32)
    idx = np.asarray(pooling_indices).astype(np.int64)
    T = int(np.asarray(target_length))

    t, seg, valid, segc, lidx, lens = _segment_structure(idx, T)

    order = np.argsort(-lens, kind="stable")
    rank_of_seg = np.empty(N, dtype=np.int64)
    rank_of_seg[order] = np.arange(N)
    N_l = (lens[None, :] > np.arange(L)[:, None]).sum(axis=1)

    Cs, slot_map = _plan(N_l)
    P = len(Cs)

    nc = _build_program(Cs)

    # host-side gathered inputs, fp16
    Xg = np.ascontiguousarray(pooled.transpose(2, 0, 1)[:, :, order]).astype(
        np.float16
    )  # (D, B, N) sorted columns
    Wt16 = np.ascontiguousarray(Wf.transpose(0, 2, 1)).astype(np.float16)  # (L,D,D) .T

    in_maps = []
    for c in range(NCORES):
        wt_c = np.zeros((P, D, D), dtype=np.float16)
        im = {}
        for p in range(P):
            xp = np.zeros((D, B, Cs[p]), dtype=np.float16)
            piece = slot_map[c][p]
            if piece is not None:
                l, c0, cnt = piece
                xp[:, :, :cnt] = Xg[:, :, c0 : c0 + cnt]
                wt_c[p] = Wt16[l]
            im[f"x{p}"] = xp
        im["wt"] = wt_c
        in_maps.append(im)

    kwargs = {}
    if _trace:
        _install_ntff_hook()
        kwargs = dict(trace=True)
    res = run_bass_kernel_spmd(nc, in_maps, core_ids=list(range(NCORES)), **kwargs)
    results = res.results

    # per-(l, col-rank) -> (core, phase, j) maps
    maxN = int(N_l.max()) if L else 0
    core_of = np.full((L, max(maxN, 1)), -1, dtype=np.int32)
    phase_of = np.zeros((L, max(maxN, 1)), dtype=np.int32)
    j_of = np.zeros((L, max(maxN, 1)), dtype=np.int32)
    for c in range(NCORES):
        for p in range(P):
            piece = slot_map[c][p]
            if piece is None:
                continue
            l, c0, cnt = piece
            core_of[l, c0 : c0 + cnt] = c
            phase_of[l, c0 : c0 + cnt] = p
            j_of[l, c0 : c0 + cnt] = np.arange(cnt)

    Dout = Wf.shape[1]
    out = np.zeros((B, T, Dout), dtype=np.float32)
    tv = t[valid]
    l_t = lidx[valid]
    r_t = rank_of_seg[segc[valid]]
    ct = core_of[l_t, r_t]
    pt = phase_of[l_t, r_t]
    jt = j_of[l_t, r_t]
    assert (ct >= 0).all(), "uncovered (l, col) in assignment"

    for p in range(P):
        sel = pt == p
        if not sel.any():
            continue
        Yp = np.stack([results[c][f"y{p}"] for c in range(NCORES)])  # (8,B,D,C_p)
        out[:, tv[sel], :] = Yp[ct[sel], :, :, jt[sel]].transpose(1, 0, 2)

    if _trace:
        kernel._last_exec_time_ns = res.exec_time_ns
        kernel._last_results = res
    return out


# revision 30
# speedup vs baseline: 1.0828x; 1.0828x over previous
"""MultiLinearUpsampling Trainium2 kernel.

Problem: out[b, t, :] = W[lidx[t]] @ pooled[b, segc[t], :]  (zero for invalid t)
where segc/lidx derive from sorted pooling_indices (ragged segments).

Strategy
--------
Host computes the segment structure.  Only sum_l N_l matvecs are unique
per batch (N_l = #segments with len > l; positions past offset L-1 in a
segment reuse the l=L-1 result).  The device runs one SPMD program on 8
cores: P phases, phase p = one stationary weight plane (per-core data)
applied to C_p activation columns (per-core data, host-gathered).  A
small packing optimizer cuts the 16 linears' column sets into <=8
pieces per phase to minimize sum(C_p) (the per-core compute).  Matmuls
run in fp16 (full PE rate, ~3e-4 rel err for this data, half the DMA of
fp32); accumulation is fp32 in PSUM and outputs are fp32.  The host
scatters computed vectors to their t positions (including the l=15 tail
replication) and zero-fills invalid t.
"""

from contextlib import ExitStack

import numpy as np

import concourse.bass as bass  # noqa: F401  (bass types via bacc)
import concourse.mybir as mybir
import concourse.tile as tile
from concourse import bacc
from concourse.bass_utils import run_bass_kernel_spmd

F32 = mybir.dt.float32
F16 = mybir.dt.float16

B = 8          # batch (each core sees all batches)
N = 512        # segments
D = 1024       # D_in == D_out
L = 16         # linears
NCORES = 8
KC = 8         # contraction chunks of 128
MC = 8         # output-dim chunks of 128


# ---------------------------------------------------------------------------
# packing: choose phase sizes + piece assignment
# ---------------------------------------------------------------------------

def _combo_dp(sizes, Cs):
    """Assign each item (size) a piece-count vector over phase capacities Cs
    (max 8 pieces per phase) minimizing nothing fancy -- returns None if
    infeasible, else list of per-item count tuples."""
    P = len(Cs)
    items = list(sizes)
    combos_per_item = []
    for sz in items:
        combos = []
        max_counts = [min(8, -(-sz // c) if c else 0) for c in Cs]
        # enumerate small count vectors (total pieces <= 4)
        def rec(i, vec, cap):
            if sum(vec) > 4:
                return
            if i == P:
                if cap >= sz and sum(vec) > 0:
                    combos.append(tuple(vec))
                return
            for n in range(0, min(max_counts[i], 4) + 1):
                rec(i + 1, vec + [n], cap + n * Cs[i])

        rec(0, [], 0)
        if sz > 0 and not combos:
            return None
        combos_per_item.append(combos if sz > 0 else [tuple([0] * P)])

    # DP over cumulative piece counts in phases 0..P-2, minimize last phase
    from functools import lru_cache

    states = {tuple([0] * (P - 1)): 0}
    choice = []
    for combos in combos_per_item:
        nstates = {}
        back = {}
        for st, lastc in states.items():
            for cb in combos:
                nst = tuple(st[i] + cb[i] for i in range(P - 1))
                if any(v > 8 for v in nst):
                    continue
                nl = lastc + cb[P - 1]
                if nl > 8:
                    continue
                if nst not in nstates or nl < nstates[nst]:
                    nstates[nst] = nl
                    back[nst] = (st, cb)
        if not nstates:
            return None
        choice.append(back)
        states = nstates

    st = min(states, key=lambda s: states[s])
    picks = [None] * len(items)
    for i in range(len(items) - 1, -1, -1):
        st_prev, cb = choice[i][st]
        picks[i] = cb
        st = st_prev
    return picks


def _plan(N_l):
    """Return (Cs, slot_map): phase sizes and slot_map[c][p] =
    (l, col_start, cnt) or None."""
    order_l = np.argsort(-np.asarray(N_l), kind="stable")
    Ns = [int(N_l[i]) for i in order_l]
    total = sum(Ns)
    if total == 0:
        return [2], [[None] for _ in range(NCORES)]

    best = None  # (sumC, Cs, picks)

    def consider(Cs):
        nonlocal best
        Cs = [int(c) for c in Cs if c > 0]
        if not Cs:
            return
        if best is not None and sum(Cs) >= best[0]:
            return
        picks = _combo_dp(Ns, Cs)
        if picks is not None:
            best = (sum(Cs), Cs, picks)

    # baseline: heads unsplit at N(0), tails at N(8)
    c2 = Ns[8] if len(Ns) > 8 else 0
    consider([Ns[0], c2] if c2 else [Ns[0]])

    # precomputed optima for the benchmark's N_l (verified for the actual
    # data by the feasibility DP; harmless no-ops when infeasible)
    consider([214, 170, 110])
    consider([158, 132, 94, 80])

    if c2:
        head = Ns[:8]
        # family: tail phase at N(8); two head phases (X, Y) searched
        hi = head[0]
        for X in range(max(hi // 2, 64), hi + 1, 4):
            # minimal Y so every head item fits in <=3 pieces approx
            for Y in range(16, X + 1, 4):
                if 8 * (X + Y) < sum(head):
                    continue
                if best is not None and X + Y + c2 >= best[0]:
                    continue
                picks = _combo_dp(head, [X, Y])
                if picks is None:
                    continue
                # combine with tail phase
                consider([X, Y, c2])
                break  # smallest feasible Y for this X

    Cs = best[1]
    picks = best[2]
    # build slot map: phase -> list of pieces
    P = len(Cs)
    phase_pieces = [[] for _ in range(P)]
    for idx, l in enumerate(order_l):
        sz = Ns[idx]
        if sz == 0:
            continue
        pos = 0
        cb = picks[idx] if len(picks[idx]) == P else tuple(
            list(picks[idx]) + [0] * (P - len(picks[idx]))
        )
        for p in range(P):
            for _ in range(cb[p]):
                cnt = min(Cs[p], sz - pos)
                if cnt <= 0:
                    continue
                phase_pieces[p].append((int(l), pos, cnt))
                pos += cnt
        assert pos >= sz, f"l={l} not covered: {pos}/{sz}"

    slot_map = [[None] * P for _ in range(NCORES)]
    for p in range(P):
        assert len(phase_pieces[p]) <= NCORES, (p, phase_pieces[p])
        for c, piece in enumerate(phase_pieces[p]):
            slot_map[c][p] = piece
    return Cs, slot_map


# ---------------------------------------------------------------------------
# device program
# ---------------------------------------------------------------------------

def _build_program(Cs):
    """Inputs: xt (D, B, CT) f16, wt (P, D, D) f16 (wt[p] = W-plane.T).
    Outputs: y{p} (D, B, C_p) f32."""
    nc = bacc.Bacc("TRN2", target_bir_lowering=False, debug=False)
    P = len(Cs)
    CT = sum(Cs)

    xs = [
        nc.dram_tensor(f"x{p}", (D, B, C), F16, kind="ExternalInput")
        for p, C in enumerate(Cs)
    ]
    wt = nc.dram_tensor("wt", (P, D, D), F16, kind="ExternalInput")
    ys = [
        nc.dram_tensor(f"y{p}", (B, D, C), F32, kind="ExternalOutput")
        for p, C in enumerate(Cs)
    ]

    # (kp, kc, b*c) views: per-k rows are contiguous B*C_p runs -> 2D DMAs
    xs_r = [
        x.ap().rearrange("(kc kp) b n -> kp kc (b n)", kp=128) for x in xs
    ]
    wt_r = wt.ap().rearrange("p (kc kp) m -> kp p kc m", kp=128)

    # process phases smallest-first: the first phase's inputs arrive
    # quickly, later phases' inputs stream in behind its compute
    order_p = sorted(range(P), key=lambda p: Cs[p])

    with tile.TileContext(nc) as tc, ExitStack() as ctx:
        wpool = ctx.enter_context(tc.tile_pool(name="w", bufs=1))
        xpool = ctx.enter_context(tc.tile_pool(name="x", bufs=1))
        opool = ctx.enter_context(tc.tile_pool(name="o", bufs=3))
        ppool = ctx.enter_context(tc.tile_pool(name="ps", bufs=6, space="PSUM"))

        # resident inputs, emitted in consumption order (few large DMAs:
        # each dma_start costs ~600ns of sequencer issue time regardless
        # of size, so instruction count matters more than granularity)
        wtiles = {}
        xtiles = {}
        for p in order_p:
            C = Cs[p]
            for k in range(KC):
                wtiles[p, k] = wpool.tile(
                    [128, D], F16, tag=f"w{p}_{k}", name=f"w{p}_{k}"
                )
                nc.sync.dma_start(wtiles[p, k][:], wt_r[:, p, k])
                xtiles[p, k] = xpool.tile(
                    [128, B, C], F16, tag=f"x{p}_{k}", name=f"x{p}_{k}"
                )
                nc.sync.dma_start(
                    xtiles[p, k][:].rearrange("kp b n -> kp (b n)"), xs_r[p][:, k]
                )

        for p in order_p:
            C = Cs[p]
            g = max(1, min(B, 512 // C))
            for b0 in range(0, B, g):
                gg = min(g, B - b0)
                ot = opool.tile([128, MC, gg, C], F32, tag="o")
                for m in range(MC):
                    ps = ppool.tile([128, gg, C], F32, tag="ps")
                    for k in range(KC):
                        nc.tensor.matmul(
                            ps[:],
                            wtiles[p, k][:, m * 128 : (m + 1) * 128],
                            xtiles[p, k][:, b0 : b0 + gg, :],
                            start=(k == 0),
                            stop=(k == KC - 1),
                        )
                    nc.vector.tensor_copy(ot[:, m], ps[:])
                # one output DMA per batch row (GpSimd queue: keeps the
                # Sync sequencer free for input issue)
                for bi in range(gg):
                    nc.gpsimd.dma_start(
                        ys[p][b0 + bi].rearrange("(m kp) c -> kp m c", kp=128),
                        ot[:, :, bi, :],
                    )

    nc.compile()
    return nc


# ---------------------------------------------------------------------------
# host wrapper
# ---------------------------------------------------------------------------

def _segment_structure(idx, T):
    t = np.arange(T)
    seg = np.searchsorted(idx, t, side="left")
    valid = seg < N
    segc = np.clip(seg, 0, N - 1)
    start = np.where(segc > 0, idx[np.maximum(segc - 1, 0)] + 1, 0)
    lidx = np.minimum(t - start, L - 1).astype(np.int64)
    lens = np.bincount(segc[valid], minlength=N)
    return t, seg, valid, segc, lidx, lens


def _install_ntff_hook():
    """Profiling-only: register the axon NTFF profile hook (dev use)."""
    import sys
    import types

    try:
        import antenv

        if "antenv.axon_hooks" not in sys.modules:
            mod = types.ModuleType("antenv.axon_hooks")
            holder = [None]
            mod.set_axon_ntff_profile_hook = lambda h: holder.__setitem__(0, h)
            mod.get_axon_ntff_profile_hook = lambda: holder[0]
            sys.modules["antenv.axon_hooks"] = mod
            antenv.axon_hooks = mod
            from trn_agent_boot.trn_boot import _ntff_profile_via_ctypes

            mod.set_axon_ntff_profile_hook(
                _ntff_profile_via_ctypes("/opt/axon/libaxon_pjrt.so")
            )
    except Exception as e:
        print(f"NTFF hook install failed: {e}")


def kernel(pooled_vectors, W, pooling_indices, target_length, _trace=False):
    pooled = np.asarray(pooled_vectors, dtype=np.float32)
    Wf = np.asarray(W, dtype=np.float32)
    idx = np.asarray(pooling_indices).astype(np.int64)
    T = int(np.asarray(target_length))

    t, seg, valid, segc, lidx, lens = _segment_structure(idx, T)

    order = np.argsort(-lens, kind="stable")
    rank_of_seg = np.empty(N, dtype=np.int64)
    rank_of_seg[order] = np.arange(N)
    N_l = (lens[None, :] > np.arange(L)[:, None]).sum(axis=1)

    Cs, slot_map = _plan(N_l)
    P = len(Cs)

    nc = _build_program(Cs)

    # host-side gathered inputs, fp16
    Xg = np.ascontiguousarray(pooled.transpose(2, 0, 1)[:, :, order]).astype(
        np.float16
    )  # (D, B, N) sorted columns
    Wt16 = np.ascontiguousarray(Wf.transpose(0, 2, 1)).astype(np.float16)  # (L,D,D) .T

    in_maps = []
    for c in range(NCORES):
        wt_c = np.zeros((P, D, D), dtype=np.float16)
        im = {}
        for p in range(P):
            xp = np.zeros((D, B, Cs[p]), dtype=np.float16)
            piece = slot_map[c][p]
            if piece is not None:
                l, c0, cnt = piece
                xp[:, :, :cnt] = Xg[:, :, c0 : c0 + cnt]
                wt_c[p] = Wt16[l]
            im[f"x{p}"] = xp
        im["wt"] = wt_c
        in_maps.append(im)

    kwargs = {}
    if _trace:
        _install_ntff_hook()
        kwargs = dict(trace=True)
    res = run_bass_kernel_spmd(nc, in_maps, core_ids=list(range(NCORES)), **kwargs)
    results = res.results

    # per-(l, col-rank) -> (core, phase, j) maps
    maxN = int(N_l.max()) if L else 0
    core_of = np.full((L, max(maxN, 1)), -1, dtype=np.int32)
    phase_of = np.zeros((L, max(maxN, 1)), dtype=np.int32)
    j_of = np.zeros((L, max(maxN, 1)), dtype=np.int32)
    for c in range(NCORES):
        for p in range(P):
            piece = slot_map[c][p]
            if piece is None:
                continue
            l, c0, cnt = piece
            core_of[l, c0 : c0 + cnt] = c
            phase_of[l, c0 : c0 + cnt] = p
            j_of[l, c0 : c0 + cnt] = np.arange(cnt)

    Dout = Wf.shape[1]
    out = np.zeros((B, T, Dout), dtype=np.float32)
    tv = t[valid]
    l_t = lidx[valid]
    r_t = rank_of_seg[segc[valid]]
    ct = core_of[l_t, r_t]
    pt = phase_of[l_t, r_t]
    jt = j_of[l_t, r_t]
    assert (ct >= 0).all(), "uncovered (l, col) in assignment"

    for p in range(P):
        sel = pt == p
        if not sel.any():
            continue
        Yp = np.stack([results[c][f"y{p}"] for c in range(NCORES)])  # (8,B,D,C_p)
        out[:, tv[sel], :] = Yp[ct[sel], :, :, jt[sel]].transpose(1, 0, 2)

    if _trace:
        kernel._last_exec_time_ns = res.exec_time_ns
        kernel._last_results = res
    return out
